# revision 1
# baseline (speedup 1.0000x reference)
"""CNF GNN layer (l2c segment-mean + c2l segment-mean + clause self-transform)
on 8 Trainium2 NeuronCores.

Strategy:
  - Clauses (then literals) are degree-sorted and packed into 128-row blocks;
    blocks are dealt round-robin to the 8 cores so every core gets the same
    program shape (SPMD) and a balanced edge count.
  - Per block, each "slot" column j is one [128,1] indirect-DMA gather (one
    table row per partition) -- the only indirect form this toolchain executes
    correctly. Segment-sum is then K static in-place DVE adds, mean is a
    per-partition scalar multiply by host-precomputed 1/deg.
  - Launch 1 (clause side): gather feat_literal rows, segment-mean, then
    h_clause = mean @ W_l2c + b (PE transpose + matmul), cembs = relu,
    Wh_c2l = cembs @ W_c2l + b_c2l written out; plus the independent
    h2_clause = relu(feat_clause @ W_l2c + b) stream.
  - Host gathers the per-core Wh_c2l partitions into one table (the "halo
    exchange"), then Launch 2 (literal side): gather Wh_c2l rows per edge,
    segment-mean -> h_lit. No matmul needed (bias folds through the mean).
"""
import time

import numpy as np

import concourse.bass as bass
import concourse.mybir as mybir
import concourse.tile as tile
from concourse.masks import make_identity

# ----------------------------------------------------------------------------
# Workarounds for this container's walrus (see dev notes):
#  - at most ONE semaphore wait per instruction -> split extras onto NOPs
#  - the Tile tail drain carries many waits -> same treatment
# ----------------------------------------------------------------------------
import bass_rust as _bass_rust
from concourse.vector_clock import ScopedClock as _ScopedClock

_nop_counter = [0]


def _make_nop(engine):
    _nop_counter[0] += 1
    nop = _bass_rust.InstNoOp(name=f"I-wsplit-{_nop_counter[0]}", ins=[], outs=[])
    nop.engine = engine
    return nop


def _split_multiwaits(nc):
    for fn in nc.m.functions:
        for b in fn.blocks:
            insts = b.instructions
            i = 0
            while i < len(insts):
                inst = insts[i]
                si = inst.sync_info
                if si is not None and si.on_wait is not None and len(si.on_wait) > 1:
                    waits = list(si.on_wait)
                    si.on_wait = waits[-1:]
                    for w in waits[:-1]:
                        nop = _make_nop(inst.engine)
                        nop.sync_info = mybir.SyncInfo(on_wait=[w], on_update=[])
                        insts.insert(i, nop)
                        i += 1
                i += 1


def _patched_drain_and_barrier(self, tick_clock, wait_clock):
    nc = self.nc
    carrier = nc.sync.nop(nofuse=True)
    wait_clock.add_sem_waits(carrier.ins, _ScopedClock({None: tick_clock.global_clock}))
    si = carrier.ins.sync_info
    waits = list(si.on_wait) if si is not None else []
    if si is not None and len(waits) > 1:
        si.on_wait = waits[:1]
        for i in range(1, len(waits)):
            extra = nc.sync.nop(nofuse=True)
            extra.ins.sync_info = mybir.SyncInfo(on_wait=waits[i : i + 1], on_update=[])
    nc.sync.drain()
    nc.all_engine_barrier()
    popped = nc._tile_sem_poison_stack.pop()
    assert popped is self._sem_poison
    nc.clear_and_free_semaphores(list(self.sems.allocated().values()))
    nc.all_engine_barrier()


tile.TileContext._drain_and_barrier = _patched_drain_and_barrier

# ----------------------------------------------------------------------------
# PJRT SPMD runner (axon path); builds the executable once.
# ----------------------------------------------------------------------------
import jax
from jax.sharding import Mesh, NamedSharding, PartitionSpec
from jax.experimental.shard_map import shard_map
from concourse.bass2jax import _bass_exec_p, install_neuronx_cc_hook, partition_id_tensor


class _SpmdRunner:
    def __init__(self, nc, n_cores=8):
        install_neuronx_cc_hook()
        self.nc = nc
        self.n_cores = n_cores
        partition_name = nc.partition_id_tensor.name if nc.partition_id_tensor else None
        in_names, out_names, out_avals, zero_outs = [], [], [], []
        for alloc in nc.m.functions[0].allocations:
            if not isinstance(alloc, mybir.MemoryLocationSet):
                continue
            name = alloc.memorylocations[0].name
            if alloc.kind == "ExternalInput":
                if name != partition_name:
                    in_names.append(name)
            elif alloc.kind == "ExternalOutput":
                out_names.append(name)
                shape = tuple(alloc.tensor_shape)
                dtype = mybir.dt.np(alloc.dtype)
                out_avals.append(jax.core.ShapedArray(shape, dtype))
                zero_outs.append(np.zeros(shape, dtype))
        self.in_names = in_names
        self.out_names = out_names
        self.zero_outs = zero_outs
        n_params = len(in_names)
        n_outs = len(out_avals)
        all_in_names = in_names + out_names
        if partition_name is not None:
            all_in_names.append(partition_name)

        def _body(*args):
            operands = list(args)
            if partition_name is not None:
                operands.append(partition_id_tensor())
            outs = _bass_exec_p.bind(
                *operands,
                out_avals=tuple(out_avals),
                in_names=tuple(all_in_names),
                out_names=tuple(out_names),
                lowering_input_output_aliases=(),
                sim_require_finite=True,
                sim_require_nnan=True,
                nc=nc,
            )
            return tuple(outs)

        devices = jax.devices()[:n_cores]
        self.mesh = Mesh(np.asarray(devices), ("core",))
        in_specs = (PartitionSpec("core"),) * (n_params + n_outs)
        out_specs = (PartitionSpec("core"),) * n_outs
        self.fn = jax.jit(
            shard_map(_body, mesh=self.mesh, in_specs=in_specs,
                      out_specs=out_specs, check_rep=False),
            keep_unused=True,
        )
        self._dev_args = None

    def stage_inputs(self, in_maps):
        n = self.n_cores
        per_core = [[np.asarray(m[name]) for name in self.in_names] for m in in_maps]
        concat_in = [
            np.concatenate([per_core[c][i] for c in range(n)], axis=0)
            for i in range(len(self.in_names))
        ]
        concat_outs = [
            np.zeros((z.shape[0] * n,) + z.shape[1:], z.dtype) for z in self.zero_outs
        ]
        shard = NamedSharding(self.mesh, PartitionSpec("core"))
        self._dev_args = [jax.device_put(a, shard) for a in concat_in + concat_outs]

    def run(self):
        outs = self.fn(*self._dev_args)
        jax.block_until_ready(outs)
        return outs

    def run_numpy(self):
        outs = self.run()
        n = self.n_cores
        results = [dict() for _ in range(n)]
        for name, arr in zip(self.out_names, outs):
            arr = np.asarray(arr)
            per = arr.shape[0] // n
            for c in range(n):
                results[c][name] = arr[c * per : (c + 1) * per]
        return results


# ----------------------------------------------------------------------------
# Problem constants (hardcoded per the task contract)
# ----------------------------------------------------------------------------
N_LIT = 100000
N_CLAUSE = 300000
N_EDGES = 3000000
D = 64
P = 128
NCORES = 8

NBLK_A = -(-N_CLAUSE // (P * NCORES))          # 293 clause blocks per core
NPAD_A = NBLK_A * P * NCORES                   # 300032
NBLK_B = -(-N_LIT // (P * NCORES))             # 98 literal blocks per core
NPAD_B = NBLK_B * P * NCORES                   # 100352


def _block_layout(idx_dst, idx_src, n_dst, npad, nblk):
    """Degree-sort destinations, pack into P-row blocks dealt round-robin to
    cores; return per-edge (core, partition, block, slot) plus block slot
    widths (uniform across cores) and per-dst storage mapping."""
    deg = np.bincount(idx_dst, minlength=n_dst)
    deg_ext = np.concatenate([deg, np.zeros(npad - n_dst, np.int64)])
    order = np.argsort(deg_ext, kind="stable")          # ascending degree
    pos = np.empty(npad, np.int64)
    pos[order] = np.arange(npad)
    g = pos // P                    # global block id per dst
    p_of = pos % P
    core_of = g % NCORES
    b_of = g // NCORES
    # uniform-across-cores slot width per local block index b
    deg_sorted = deg_ext[order].reshape(-1, P)          # [nblk*NCORES, P]
    kg = deg_sorted.max(axis=1)                         # per global block
    kb = np.maximum(kg.reshape(nblk, NCORES).max(axis=1), 1).astype(np.int64)
    col0 = np.concatenate([[0], np.cumsum(kb)])         # [nblk+1]
    # per-edge slot j within its destination
    order_e = np.argsort(idx_dst, kind="stable")
    sorted_d = idx_dst[order_e]
    first = np.ones(len(idx_dst), bool)
    first[1:] = sorted_d[1:] != sorted_d[:-1]
    gstart = np.flatnonzero(first)
    run_id = np.cumsum(first) - 1
    j_sorted = np.arange(len(idx_dst)) - gstart[run_id]
    j_e = np.empty(len(idx_dst), np.int64)
    j_e[order_e] = j_sorted
    ce = core_of[idx_dst]
    pe = p_of[idx_dst]
    cole = col0[b_of[idx_dst]] + j_e
    return {
        "deg": deg_ext, "core_of": core_of, "p_of": p_of, "b_of": b_of,
        "kb": kb, "col0": col0, "ncols": int(col0[-1]),
        "edge_core": ce, "edge_p": pe, "edge_col": cole,
    }


def _build_launch1(kb, col0, ncols, n_table_rows):
    nc = bass.Bass()
    lit = nc.declare_dram_parameter("lit", [n_table_rows, D], mybir.dt.float32, isOutput=False)
    idxa = nc.declare_dram_parameter("idxa", [P, ncols], mybir.dt.int32, isOutput=False)
    inva = nc.declare_dram_parameter("inva", [P, NBLK_A], mybir.dt.float32, isOutput=False)
    fc = nc.declare_dram_parameter("fc", [P * NBLK_A, D], mybir.dt.float32, isOutput=False)
    Wl = nc.declare_dram_parameter("Wl", [D, D], mybir.dt.float32, isOutput=False)
    Wc = nc.declare_dram_parameter("Wc", [D, D], mybir.dt.float32, isOutput=False)
    blb = nc.declare_dram_parameter("blb", [P, D], mybir.dt.float32, isOutput=False)
    bcb = nc.declare_dram_parameter("bcb", [P, D], mybir.dt.float32, isOutput=False)
    wh = nc.declare_dram_parameter("wh", [P * NBLK_A, D], mybir.dt.float32, isOutput=True)
    h2 = nc.declare_dram_parameter("h2", [P * NBLK_A, D], mybir.dt.float32, isOutput=True)

    kmax = int(kb.max())
    CB = 16  # blocks per output chunk
    fc_v = fc[:].rearrange("(p b) d -> p b d", p=P)
    wh_v = wh[:].rearrange("(p b) d -> p b d", p=P)
    h2_v = h2[:].rearrange("(p b) d -> p b d", p=P)

    with tile.TileContext(nc) as tc:
        with (
            tc.tile_pool(name="msg", bufs=4) as msgp,
            tc.tile_pool(name="stage", bufs=2) as stagep,
            tc.tile_pool(name="work", bufs=4) as workp,
            tc.tile_pool(name="cst", bufs=1) as cstp,
            tc.tile_pool(name="psum", bufs=4, space="PSUM") as psump,
        ):
            ident = cstp.tile([P, P], mybir.dt.float32)
            make_identity(nc, ident[:])
            Wl_t = cstp.tile([D, D], mybir.dt.float32)
            nc.sync.dma_start(out=Wl_t[:], in_=Wl[:])
            Wc_t = cstp.tile([D, D], mybir.dt.float32)
            nc.sync.dma_start(out=Wc_t[:], in_=Wc[:])
            blb_t = cstp.tile([P, D], mybir.dt.float32)
            nc.sync.dma_start(out=blb_t[:], in_=blb[:])
            bcb_t = cstp.tile([P, D], mybir.dt.float32)
            nc.sync.dma_start(out=bcb_t[:], in_=bcb[:])
            idx_t = cstp.tile([P, ncols], mybir.dt.int32)
            nc.sync.dma_start(out=idx_t[:], in_=idxa[:])
            inv_t = cstp.tile([P, NBLK_A], mybir.dt.float32)
            nc.sync.dma_start(out=inv_t[:], in_=inva[:])

            for c0 in range(0, NBLK_A, CB):
                c1 = min(c0 + CB, NBLK_A)
                nb = c1 - c0
                wh_s = stagep.tile([P, CB, D], mybir.dt.float32, tag="whs")
                h2_s = stagep.tile([P, CB, D], mybir.dt.float32, tag="h2s")
                fc_t = stagep.tile([P, CB, D], mybir.dt.float32, tag="fct")
                nc.sync.dma_start(out=fc_t[:, :nb, :], in_=fc_v[:, c0:c1, :])
                for b in range(c0, c1):
                    bi = b - c0
                    K = int(kb[b])
                    base = int(col0[b])
                    msg = msgp.tile([P, kmax, D], mybir.dt.float32, tag="msg")
                    for j in range(K):
                        nc.gpsimd.indirect_dma_start(
                            out=msg[:, j, :],
                            out_offset=None,
                            in_=lit[:],
                            in_offset=bass.IndirectOffsetOnAxis(
                                ap=idx_t[:, base + j : base + j + 1], axis=0
                            ),
                        )
                    acc = msg[:, 0, :]
                    for j in range(1, K):
                        nc.vector.tensor_add(acc, acc, msg[:, j, :])
                    nc.vector.tensor_scalar_mul(acc, acc, inv_t[:, b : b + 1])
                    # mean @ Wl + bl -> relu -> @ Wc + bc
                    pt = psump.tile([D, P], mybir.dt.float32, tag="pt")
                    nc.tensor.transpose(out=pt[:], in_=acc, identity=ident[:])
                    st = workp.tile([D, P], mybir.dt.float32, tag="st")
                    nc.vector.tensor_copy(st[:], pt[:])
                    mm1 = psump.tile([P, D], mybir.dt.float32, tag="mm")
                    nc.tensor.matmul(out=mm1[:], lhsT=st[:], rhs=Wl_t[:], start=True, stop=True)
                    cem = workp.tile([P, D], mybir.dt.float32, tag="cem")
                    nc.vector.tensor_add(cem[:], mm1[:], blb_t[:])
                    nc.scalar.activation(cem[:], cem[:], mybir.ActivationFunctionType.Relu)
                    pt2 = psump.tile([D, P], mybir.dt.float32, tag="pt")
                    nc.tensor.transpose(out=pt2[:], in_=cem[:], identity=ident[:])
                    st2 = workp.tile([D, P], mybir.dt.float32, tag="st")
                    nc.vector.tensor_copy(st2[:], pt2[:])
                    mm2 = psump.tile([P, D], mybir.dt.float32, tag="mm")
                    nc.tensor.matmul(out=mm2[:], lhsT=st2[:], rhs=Wc_t[:], start=True, stop=True)
                    nc.vector.tensor_add(wh_s[:, bi, :], mm2[:], bcb_t[:])
                    # h2 = relu(fc @ Wl + bl)
                    ptf = psump.tile([D, P], mybir.dt.float32, tag="pt")
                    nc.tensor.transpose(out=ptf[:], in_=fc_t[:, bi, :], identity=ident[:])
                    stf = workp.tile([D, P], mybir.dt.float32, tag="st")
                    nc.vector.tensor_copy(stf[:], ptf[:])
                    mmf = psump.tile([P, D], mybir.dt.float32, tag="mm")
                    nc.tensor.matmul(out=mmf[:], lhsT=stf[:], rhs=Wl_t[:], start=True, stop=True)
                    h2t = workp.tile([P, D], mybir.dt.float32, tag="cem")
                    nc.vector.tensor_add(h2t[:], mmf[:], blb_t[:])
                    nc.scalar.activation(h2_s[:, bi, :], h2t[:], mybir.ActivationFunctionType.Relu)
                nc.sync.dma_start(out=wh_v[:, c0:c1, :], in_=wh_s[:, :nb, :])
                nc.sync.dma_start(out=h2_v[:, c0:c1, :], in_=h2_s[:, :nb, :])
    _split_multiwaits(nc)
    return nc


def _build_launch2(kb, col0, ncols, n_table_rows):
    nc = bass.Bass()
    whf = nc.declare_dram_parameter("whf", [n_table_rows, D], mybir.dt.float32, isOutput=False)
    idxb = nc.declare_dram_parameter("idxb", [P, ncols], mybir.dt.int32, isOutput=False)
    invb = nc.declare_dram_parameter("invb", [P, NBLK_B], mybir.dt.float32, isOutput=False)
    hl = nc.declare_dram_parameter("hl", [P * NBLK_B, D], mybir.dt.float32, isOutput=True)

    kmax = int(kb.max())
    CB = 8
    hl_v = hl[:].rearrange("(p b) d -> p b d", p=P)

    with tile.TileContext(nc) as tc:
        with (
            tc.tile_pool(name="msg", bufs=3) as msgp,
            tc.tile_pool(name="stage", bufs=2) as stagep,
            tc.tile_pool(name="cst", bufs=1) as cstp,
        ):
            idx_t = cstp.tile([P, ncols], mybir.dt.int32)
            nc.sync.dma_start(out=idx_t[:], in_=idxb[:])
            inv_t = cstp.tile([P, NBLK_B], mybir.dt.float32)
            nc.sync.dma_start(out=inv_t[:], in_=invb[:])
            for c0 in range(0, NBLK_B, CB):
                c1 = min(c0 + CB, NBLK_B)
                nb = c1 - c0
                hl_s = stagep.tile([P, CB, D], mybir.dt.float32, tag="hls")
                for b in range(c0, c1):
                    bi = b - c0
                    K = int(kb[b])
                    base = int(col0[b])
                    msg = msgp.tile([P, kmax, D], mybir.dt.float32, tag="msg")
                    for j in range(K):
                        nc.gpsimd.indirect_dma_start(
                            out=msg[:, j, :],
                            out_offset=None,
                            in_=whf[:],
                            in_offset=bass.IndirectOffsetOnAxis(
                                ap=idx_t[:, base + j : base + j + 1], axis=0
                            ),
                        )
                    acc = msg[:, 0, :]
                    for j in range(1, K):
                        nc.vector.tensor_add(acc, acc, msg[:, j, :])
                    nc.vector.tensor_scalar_mul(hl_s[:, bi, :], acc, inv_t[:, b : b + 1])
                nc.sync.dma_start(out=hl_v[:, c0:c1, :], in_=hl_s[:, :nb, :])
    _split_multiwaits(nc)
    return nc


LAST_STATS = {}
_RUNNERS = {}


def kernel(feat_literal, feat_clause, lit_idx, clause_idx, W_l2c, b_l2c, W_c2l, b_c2l):
    feat_literal = np.asarray(feat_literal, np.float32)
    feat_clause = np.asarray(feat_clause, np.float32)
    lit_idx = np.asarray(lit_idx, np.int32)
    clause_idx = np.asarray(clause_idx, np.int32)
    W_l2c = np.asarray(W_l2c, np.float32)
    b_l2c = np.asarray(b_l2c, np.float32).reshape(1, D)
    W_c2l = np.asarray(W_c2l, np.float32)
    b_c2l = np.asarray(b_c2l, np.float32).reshape(1, D)

    t_host0 = time.time()
    # ---- phase A layout (clauses) ----
    LA = _block_layout(clause_idx, lit_idx, N_CLAUSE, NPAD_A, NBLK_A)
    ncols_a = LA["ncols"]
    ZL = N_LIT  # zero-row index in literal table
    lit_z = np.vstack([feat_literal, np.zeros((1, D), np.float32)])
    idxa = np.full((NCORES, P, ncols_a), ZL, np.int32)
    idxa[LA["edge_core"], LA["edge_p"], LA["edge_col"]] = lit_idx
    inva = np.zeros((NCORES, P, NBLK_A), np.float32)
    allc = np.arange(NPAD_A)
    degc = LA["deg"]
    inva[LA["core_of"], LA["p_of"], LA["b_of"]] = np.where(
        degc > 0, 1.0 / np.maximum(degc, 1), 0.0
    ).astype(np.float32)
    # feat_clause permuted to per-core storage order (dummies -> zeros)
    fc_all = np.zeros((NCORES, P * NBLK_A, D), np.float32)
    realc = allc[:N_CLAUSE]
    rowc = LA["p_of"] * NBLK_A + LA["b_of"]
    fc_all[LA["core_of"][realc], rowc[realc]] = feat_clause

    # ---- phase B layout (literals) ----
    LB = _block_layout(lit_idx, clause_idx, N_LIT, NPAD_B, NBLK_B)
    ncols_b = LB["ncols"]
    # per-edge gather row in the assembled wh table (+1 zero row at end)
    wh_rows = NCORES * P * NBLK_A
    ZW = wh_rows
    clause_whrow = LA["core_of"][:N_CLAUSE + 0] * 0  # placeholder
    clause_whrow = (LA["core_of"] * (P * NBLK_A) + rowc).astype(np.int32)  # [NPAD_A]
    idxb = np.full((NCORES, P, ncols_b), ZW, np.int32)
    idxb[LB["edge_core"], LB["edge_p"], LB["edge_col"]] = clause_whrow[clause_idx]
    invb = np.zeros((NCORES, P, NBLK_B), np.float32)
    degl = LB["deg"]
    invb[LB["core_of"], LB["p_of"], LB["b_of"]] = np.where(
        degl > 0, 1.0 / np.maximum(degl, 1), 0.0
    ).astype(np.float32)
    LAST_STATS["host_prep_s"] = time.time() - t_host0

    blb = np.broadcast_to(b_l2c, (P, D)).copy()
    bcb = np.broadcast_to(b_c2l, (P, D)).copy()

    # ---- launch 1 ----
    key1 = ("L1", ncols_a, tuple(LA["kb"]))
    if key1 not in _RUNNERS:
        nc1 = _build_launch1(LA["kb"], LA["col0"], ncols_a, N_LIT + 1)
        _RUNNERS[key1] = _SpmdRunner(nc1, NCORES)
    r1 = _RUNNERS[key1]
    in_maps1 = [
        {
            "lit": lit_z,
            "idxa": idxa[c],
            "inva": inva[c],
            "fc": fc_all[c],
            "Wl": W_l2c,
            "Wc": W_c2l,
            "blb": blb,
            "bcb": bcb,
        }
        for c in range(NCORES)
    ]
    r1.stage_inputs(in_maps1)
    t0 = time.time()
    res1 = r1.run_numpy()
    LAST_STATS["launch1_wall_s"] = time.time() - t0

    wh_full = np.concatenate([res1[c]["wh"] for c in range(NCORES)], axis=0)
    # deg-0 clauses: reference gives cembs=relu(0)=0 -> wh row = b_c2l
    deg0 = np.flatnonzero(degc[:N_CLAUSE] == 0)
    if len(deg0):
        wh_full[clause_whrow[deg0]] = b_c2l
    wh_z = np.vstack([wh_full, np.zeros((1, D), np.float32)])

    # ---- launch 2 ----
    key2 = ("L2", ncols_b, tuple(LB["kb"]))
    if key2 not in _RUNNERS:
        nc2 = _build_launch2(LB["kb"], LB["col0"], ncols_b, wh_rows + 1)
        _RUNNERS[key2] = _SpmdRunner(nc2, NCORES)
    r2 = _RUNNERS[key2]
    in_maps2 = [
        {"whf": wh_z, "idxb": idxb[c], "invb": invb[c]} for c in range(NCORES)
    ]
    r2.stage_inputs(in_maps2)
    t0 = time.time()
    res2 = r2.run_numpy()
    LAST_STATS["launch2_wall_s"] = time.time() - t0

    # ---- reassemble outputs ----
    h2_all = np.concatenate([res1[c]["h2"] for c in range(NCORES)], axis=0)
    h2_clause = h2_all[clause_whrow[:N_CLAUSE]]
    hl_all = np.concatenate([res2[c]["hl"] for c in range(NCORES)], axis=0)
    lit_row = (LB["core_of"] * (P * NBLK_B) + LB["p_of"] * NBLK_B + LB["b_of"]).astype(
        np.int64
    )
    h_lit = hl_all[lit_row[:N_LIT]]
    return h_lit, h2_clause


def time_launches(iters=6):
    """Re-run both staged launches to estimate device wall time (includes the
    fixed PJRT/axon dispatch overhead; subtract a null-kernel baseline for a
    cleaner kernel-only estimate)."""
    out = {}
    for key, r in _RUNNERS.items():
        ts = []
        for _ in range(iters):
            t0 = time.perf_counter()
            r.run()
            ts.append(time.perf_counter() - t0)
        out[key[0]] = min(ts)
    return out


# revision 2
# speedup vs baseline: 2.1554x; 2.1554x over previous
"""CNF GNN layer (l2c segment-mean + c2l segment-mean + clause self-transform)
on 8 Trainium2 NeuronCores.

Strategy:
  - Clauses (then literals) are degree-sorted and packed into 128-row blocks;
    blocks are dealt round-robin to the 8 cores so every core gets the same
    program shape (SPMD) and a balanced edge count.
  - Per block, each "slot" column j is one [128,1] indirect-DMA gather (one
    table row per partition) -- the only indirect form this toolchain executes
    correctly. Segment-sum is then K static in-place DVE adds, mean is a
    per-partition scalar multiply by host-precomputed 1/deg.
  - Launch 1 (clause side): gather feat_literal rows, segment-mean, then
    h_clause = mean @ W_l2c + b (PE transpose + matmul), cembs = relu,
    Wh_c2l = cembs @ W_c2l + b_c2l written out; plus the independent
    h2_clause = relu(feat_clause @ W_l2c + b) stream.
  - Host gathers the per-core Wh_c2l partitions into one table (the "halo
    exchange"), then Launch 2 (literal side): gather Wh_c2l rows per edge,
    segment-mean -> h_lit. No matmul needed (bias folds through the mean).
"""
import time

import numpy as np

import concourse.bass as bass
import concourse.mybir as mybir
import concourse.tile as tile
from concourse.masks import make_identity

# ----------------------------------------------------------------------------
# Workarounds for this container's walrus (see dev notes):
#  - at most ONE semaphore wait per instruction -> split extras onto NOPs
#  - the Tile tail drain carries many waits -> same treatment
# ----------------------------------------------------------------------------
import bass_rust as _bass_rust
from concourse.vector_clock import ScopedClock as _ScopedClock

_nop_counter = [0]


def _make_nop(engine):
    _nop_counter[0] += 1
    nop = _bass_rust.InstNoOp(name=f"I-wsplit-{_nop_counter[0]}", ins=[], outs=[])
    nop.engine = engine
    return nop


def _split_multiwaits(nc):
    for fn in nc.m.functions:
        for b in fn.blocks:
            insts = b.instructions
            i = 0
            while i < len(insts):
                inst = insts[i]
                si = inst.sync_info
                if si is not None and si.on_wait is not None and len(si.on_wait) > 1:
                    waits = list(si.on_wait)
                    si.on_wait = waits[-1:]
                    for w in waits[:-1]:
                        nop = _make_nop(inst.engine)
                        nop.sync_info = mybir.SyncInfo(on_wait=[w], on_update=[])
                        insts.insert(i, nop)
                        i += 1
                i += 1


def _patched_drain_and_barrier(self, tick_clock, wait_clock):
    nc = self.nc
    carrier = nc.sync.nop(nofuse=True)
    wait_clock.add_sem_waits(carrier.ins, _ScopedClock({None: tick_clock.global_clock}))
    si = carrier.ins.sync_info
    waits = list(si.on_wait) if si is not None else []
    if si is not None and len(waits) > 1:
        si.on_wait = waits[:1]
        for i in range(1, len(waits)):
            extra = nc.sync.nop(nofuse=True)
            extra.ins.sync_info = mybir.SyncInfo(on_wait=waits[i : i + 1], on_update=[])
    nc.sync.drain()
    nc.all_engine_barrier()
    popped = nc._tile_sem_poison_stack.pop()
    assert popped is self._sem_poison
    nc.clear_and_free_semaphores(list(self.sems.allocated().values()))
    nc.all_engine_barrier()


tile.TileContext._drain_and_barrier = _patched_drain_and_barrier

# ----------------------------------------------------------------------------
# PJRT SPMD runner (axon path); builds the executable once.
# ----------------------------------------------------------------------------
import jax
from jax.sharding import Mesh, NamedSharding, PartitionSpec
from jax.experimental.shard_map import shard_map
from concourse.bass2jax import _bass_exec_p, install_neuronx_cc_hook, partition_id_tensor


class _SpmdRunner:
    def __init__(self, nc, n_cores=8):
        install_neuronx_cc_hook()
        self.nc = nc
        self.n_cores = n_cores
        partition_name = nc.partition_id_tensor.name if nc.partition_id_tensor else None
        in_names, out_names, out_avals, zero_outs = [], [], [], []
        for alloc in nc.m.functions[0].allocations:
            if not isinstance(alloc, mybir.MemoryLocationSet):
                continue
            name = alloc.memorylocations[0].name
            if alloc.kind == "ExternalInput":
                if name != partition_name:
                    in_names.append(name)
            elif alloc.kind == "ExternalOutput":
                out_names.append(name)
                shape = tuple(alloc.tensor_shape)
                dtype = mybir.dt.np(alloc.dtype)
                out_avals.append(jax.core.ShapedArray(shape, dtype))
                zero_outs.append(np.zeros(shape, dtype))
        self.in_names = in_names
        self.out_names = out_names
        self.zero_outs = zero_outs
        n_params = len(in_names)
        n_outs = len(out_avals)
        all_in_names = in_names + out_names
        if partition_name is not None:
            all_in_names.append(partition_name)

        def _body(*args):
            operands = list(args)
            if partition_name is not None:
                operands.append(partition_id_tensor())
            outs = _bass_exec_p.bind(
                *operands,
                out_avals=tuple(out_avals),
                in_names=tuple(all_in_names),
                out_names=tuple(out_names),
                lowering_input_output_aliases=(),
                sim_require_finite=True,
                sim_require_nnan=True,
                nc=nc,
            )
            return tuple(outs)

        devices = jax.devices()[:n_cores]
        self.mesh = Mesh(np.asarray(devices), ("core",))
        in_specs = (PartitionSpec("core"),) * (n_params + n_outs)
        out_specs = (PartitionSpec("core"),) * n_outs
        self.fn = jax.jit(
            shard_map(_body, mesh=self.mesh, in_specs=in_specs,
                      out_specs=out_specs, check_rep=False),
            keep_unused=True,
        )
        self._dev_args = None

    def stage_inputs(self, in_maps):
        n = self.n_cores
        per_core = [[np.asarray(m[name]) for name in self.in_names] for m in in_maps]
        concat_in = [
            np.concatenate([per_core[c][i] for c in range(n)], axis=0)
            for i in range(len(self.in_names))
        ]
        concat_outs = [
            np.zeros((z.shape[0] * n,) + z.shape[1:], z.dtype) for z in self.zero_outs
        ]
        shard = NamedSharding(self.mesh, PartitionSpec("core"))
        self._dev_args = [jax.device_put(a, shard) for a in concat_in + concat_outs]

    def run(self):
        outs = self.fn(*self._dev_args)
        jax.block_until_ready(outs)
        return outs

    def run_numpy(self):
        outs = self.run()
        n = self.n_cores
        results = [dict() for _ in range(n)]
        for name, arr in zip(self.out_names, outs):
            arr = np.asarray(arr)
            per = arr.shape[0] // n
            for c in range(n):
                results[c][name] = arr[c * per : (c + 1) * per]
        return results


# ----------------------------------------------------------------------------
# Problem constants (hardcoded per the task contract)
# ----------------------------------------------------------------------------
N_LIT = 100000
N_CLAUSE = 300000
N_EDGES = 3000000
D = 64
P = 128
NCORES = 8

NBLK_A = -(-N_CLAUSE // (P * NCORES))          # 293 clause blocks per core
NPAD_A = NBLK_A * P * NCORES                   # 300032
NBLK_B = -(-N_LIT // (P * NCORES))             # 98 literal blocks per core
NPAD_B = NBLK_B * P * NCORES                   # 100352


def _block_layout(idx_dst, idx_src, n_dst, npad, nblk):
    """Degree-sort destinations, pack into P-row blocks dealt round-robin to
    cores; return per-edge (core, partition, block, slot) plus block slot
    widths (uniform across cores) and per-dst storage mapping."""
    deg = np.bincount(idx_dst, minlength=n_dst)
    deg_ext = np.concatenate([deg, np.zeros(npad - n_dst, np.int64)])
    order = np.argsort(deg_ext, kind="stable")          # ascending degree
    pos = np.empty(npad, np.int64)
    pos[order] = np.arange(npad)
    g = pos // P                    # global block id per dst
    p_of = pos % P
    core_of = g % NCORES
    b_of = g // NCORES
    # uniform-across-cores slot width per local block index b
    deg_sorted = deg_ext[order].reshape(-1, P)          # [nblk*NCORES, P]
    kg = deg_sorted.max(axis=1)                         # per global block
    kb = np.maximum(kg.reshape(nblk, NCORES).max(axis=1), 1).astype(np.int64)
    col0 = np.concatenate([[0], np.cumsum(kb)])         # [nblk+1]
    # per-edge slot j within its destination
    order_e = np.argsort(idx_dst, kind="stable")
    sorted_d = idx_dst[order_e]
    first = np.ones(len(idx_dst), bool)
    first[1:] = sorted_d[1:] != sorted_d[:-1]
    gstart = np.flatnonzero(first)
    run_id = np.cumsum(first) - 1
    j_sorted = np.arange(len(idx_dst)) - gstart[run_id]
    j_e = np.empty(len(idx_dst), np.int64)
    j_e[order_e] = j_sorted
    ce = core_of[idx_dst]
    pe = p_of[idx_dst]
    cole = col0[b_of[idx_dst]] + j_e
    return {
        "deg": deg_ext, "core_of": core_of, "p_of": p_of, "b_of": b_of,
        "kb": kb, "col0": col0, "ncols": int(col0[-1]),
        "edge_core": ce, "edge_p": pe, "edge_col": cole,
    }


def _build_launch1(kb, col0, ncols, n_table_rows):
    nc = bass.Bass()
    lit = nc.declare_dram_parameter("lit", [n_table_rows, D], mybir.dt.float32, isOutput=False)
    idxa = nc.declare_dram_parameter("idxa", [P, ncols], mybir.dt.int32, isOutput=False)
    inva = nc.declare_dram_parameter("inva", [P, NBLK_A], mybir.dt.float32, isOutput=False)
    fc = nc.declare_dram_parameter("fc", [P * NBLK_A, D], mybir.dt.float32, isOutput=False)
    Wl = nc.declare_dram_parameter("Wl", [D, D], mybir.dt.float32, isOutput=False)
    Wc = nc.declare_dram_parameter("Wc", [D, D], mybir.dt.float32, isOutput=False)
    blb = nc.declare_dram_parameter("blb", [P, D], mybir.dt.float32, isOutput=False)
    bcb = nc.declare_dram_parameter("bcb", [P, D], mybir.dt.float32, isOutput=False)
    wh = nc.declare_dram_parameter("wh", [P * NBLK_A, D], mybir.dt.float32, isOutput=True)
    h2 = nc.declare_dram_parameter("h2", [P * NBLK_A, D], mybir.dt.float32, isOutput=True)

    kmax = int(kb.max())
    CB = 16  # blocks per output chunk
    fc_v = fc[:].rearrange("(p b) d -> p b d", p=P)
    wh_v = wh[:].rearrange("(p b) d -> p b d", p=P)
    h2_v = h2[:].rearrange("(p b) d -> p b d", p=P)

    with tile.TileContext(nc) as tc:
        with (
            tc.tile_pool(name="msg", bufs=4) as msgp,
            tc.tile_pool(name="stage", bufs=2) as stagep,
            tc.tile_pool(name="work", bufs=4) as workp,
            tc.tile_pool(name="cst", bufs=1) as cstp,
            tc.tile_pool(name="psum", bufs=4, space="PSUM") as psump,
        ):
            ident = cstp.tile([P, P], mybir.dt.float32)
            make_identity(nc, ident[:])
            Wl_t = cstp.tile([D, D], mybir.dt.float32)
            nc.sync.dma_start(out=Wl_t[:], in_=Wl[:])
            Wc_t = cstp.tile([D, D], mybir.dt.float32)
            nc.sync.dma_start(out=Wc_t[:], in_=Wc[:])
            blb_t = cstp.tile([P, D], mybir.dt.float32)
            nc.sync.dma_start(out=blb_t[:], in_=blb[:])
            bcb_t = cstp.tile([P, D], mybir.dt.float32)
            nc.sync.dma_start(out=bcb_t[:], in_=bcb[:])
            idx_t = cstp.tile([P, ncols], mybir.dt.int32)
            nc.sync.dma_start(out=idx_t[:], in_=idxa[:])
            inv_t = cstp.tile([P, NBLK_A], mybir.dt.float32)
            nc.sync.dma_start(out=inv_t[:], in_=inva[:])

            for c0 in range(0, NBLK_A, CB):
                c1 = min(c0 + CB, NBLK_A)
                nb = c1 - c0
                wh_s = stagep.tile([P, CB, D], mybir.dt.float32, tag="whs")
                h2_s = stagep.tile([P, CB, D], mybir.dt.float32, tag="h2s")
                fc_t = stagep.tile([P, CB, D], mybir.dt.float32, tag="fct")
                nc.sync.dma_start(out=fc_t[:, :nb, :], in_=fc_v[:, c0:c1, :])
                for b in range(c0, c1):
                    bi = b - c0
                    K = int(kb[b])
                    base = int(col0[b])
                    msg = msgp.tile([P, kmax, D], mybir.dt.float32, tag="msg")
                    for j in range(K):
                        nc.gpsimd.indirect_dma_start(
                            out=msg[:, j, :],
                            out_offset=None,
                            in_=lit[:],
                            in_offset=bass.IndirectOffsetOnAxis(
                                ap=idx_t[:, base + j : base + j + 1], axis=0
                            ),
                        )
                    acc = msg[:, 0, :]
                    for j in range(1, K):
                        nc.vector.tensor_add(acc, acc, msg[:, j, :])
                    nc.vector.tensor_scalar_mul(acc, acc, inv_t[:, b : b + 1])
                    # mean @ Wl + bl -> relu -> @ Wc + bc
                    pt = psump.tile([D, P], mybir.dt.float32, tag="pt")
                    nc.tensor.transpose(out=pt[:], in_=acc, identity=ident[:])
                    st = workp.tile([D, P], mybir.dt.float32, tag="st")
                    nc.vector.tensor_copy(st[:], pt[:])
                    mm1 = psump.tile([P, D], mybir.dt.float32, tag="mm")
                    nc.tensor.matmul(out=mm1[:], lhsT=st[:], rhs=Wl_t[:], start=True, stop=True)
                    cem = workp.tile([P, D], mybir.dt.float32, tag="cem")
                    nc.vector.tensor_add(cem[:], mm1[:], blb_t[:])
                    nc.scalar.activation(cem[:], cem[:], mybir.ActivationFunctionType.Relu)
                    pt2 = psump.tile([D, P], mybir.dt.float32, tag="pt")
                    nc.tensor.transpose(out=pt2[:], in_=cem[:], identity=ident[:])
                    st2 = workp.tile([D, P], mybir.dt.float32, tag="st")
                    nc.vector.tensor_copy(st2[:], pt2[:])
                    mm2 = psump.tile([P, D], mybir.dt.float32, tag="mm")
                    nc.tensor.matmul(out=mm2[:], lhsT=st2[:], rhs=Wc_t[:], start=True, stop=True)
                    nc.vector.tensor_add(wh_s[:, bi, :], mm2[:], bcb_t[:])
                    # h2 = relu(fc @ Wl + bl)
                    ptf = psump.tile([D, P], mybir.dt.float32, tag="pt")
                    nc.tensor.transpose(out=ptf[:], in_=fc_t[:, bi, :], identity=ident[:])
                    stf = workp.tile([D, P], mybir.dt.float32, tag="st")
                    nc.vector.tensor_copy(stf[:], ptf[:])
                    mmf = psump.tile([P, D], mybir.dt.float32, tag="mm")
                    nc.tensor.matmul(out=mmf[:], lhsT=stf[:], rhs=Wl_t[:], start=True, stop=True)
                    h2t = workp.tile([P, D], mybir.dt.float32, tag="cem")
                    nc.vector.tensor_add(h2t[:], mmf[:], blb_t[:])
                    nc.scalar.activation(h2_s[:, bi, :], h2t[:], mybir.ActivationFunctionType.Relu)
                nc.sync.dma_start(out=wh_v[:, c0:c1, :], in_=wh_s[:, :nb, :])
                nc.sync.dma_start(out=h2_v[:, c0:c1, :], in_=h2_s[:, :nb, :])
    _split_multiwaits(nc)
    return nc


def _build_launch2(kb, col0, ncols, n_table_rows):
    nc = bass.Bass()
    whf = nc.declare_dram_parameter("whf", [n_table_rows, D], mybir.dt.float32, isOutput=False)
    idxb = nc.declare_dram_parameter("idxb", [P, ncols], mybir.dt.int32, isOutput=False)
    invb = nc.declare_dram_parameter("invb", [P, NBLK_B], mybir.dt.float32, isOutput=False)
    hl = nc.declare_dram_parameter("hl", [P * NBLK_B, D], mybir.dt.float32, isOutput=True)

    kmax = int(kb.max())
    CB = 8
    hl_v = hl[:].rearrange("(p b) d -> p b d", p=P)

    with tile.TileContext(nc) as tc:
        with (
            tc.tile_pool(name="msg", bufs=3) as msgp,
            tc.tile_pool(name="stage", bufs=2) as stagep,
            tc.tile_pool(name="cst", bufs=1) as cstp,
        ):
            idx_t = cstp.tile([P, ncols], mybir.dt.int32)
            nc.sync.dma_start(out=idx_t[:], in_=idxb[:])
            inv_t = cstp.tile([P, NBLK_B], mybir.dt.float32)
            nc.sync.dma_start(out=inv_t[:], in_=invb[:])
            for c0 in range(0, NBLK_B, CB):
                c1 = min(c0 + CB, NBLK_B)
                nb = c1 - c0
                hl_s = stagep.tile([P, CB, D], mybir.dt.float32, tag="hls")
                for b in range(c0, c1):
                    bi = b - c0
                    K = int(kb[b])
                    base = int(col0[b])
                    msg = msgp.tile([P, kmax, D], mybir.dt.float32, tag="msg")
                    for j in range(K):
                        nc.gpsimd.indirect_dma_start(
                            out=msg[:, j, :],
                            out_offset=None,
                            in_=whf[:],
                            in_offset=bass.IndirectOffsetOnAxis(
                                ap=idx_t[:, base + j : base + j + 1], axis=0
                            ),
                        )
                    acc = msg[:, 0, :]
                    for j in range(1, K):
                        nc.vector.tensor_add(acc, acc, msg[:, j, :])
                    nc.vector.tensor_scalar_mul(hl_s[:, bi, :], acc, inv_t[:, b : b + 1])
                nc.sync.dma_start(out=hl_v[:, c0:c1, :], in_=hl_s[:, :nb, :])
    _split_multiwaits(nc)
    return nc


LAST_STATS = {}
_RUNNERS = {}


def kernel(feat_literal, feat_clause, lit_idx, clause_idx, W_l2c, b_l2c, W_c2l, b_c2l):
    feat_literal = np.asarray(feat_literal, np.float32)
    feat_clause = np.asarray(feat_clause, np.float32)
    lit_idx = np.asarray(lit_idx, np.int32)
    clause_idx = np.asarray(clause_idx, np.int32)
    W_l2c = np.asarray(W_l2c, np.float32)
    b_l2c = np.asarray(b_l2c, np.float32).reshape(1, D)
    W_c2l = np.asarray(W_c2l, np.float32)
    b_c2l = np.asarray(b_c2l, np.float32).reshape(1, D)

    t_host0 = time.time()
    # ---- phase A layout (clauses) ----
    LA = _block_layout(clause_idx, lit_idx, N_CLAUSE, NPAD_A, NBLK_A)
    ncols_a = LA["ncols"]
    ZL = N_LIT  # zero-row index in literal table
    lit_z = np.vstack([feat_literal, np.zeros((1, D), np.float32)])
    idxa = np.full((NCORES, P, ncols_a), ZL, np.int32)
    idxa[LA["edge_core"], LA["edge_p"], LA["edge_col"]] = lit_idx
    inva = np.zeros((NCORES, P, NBLK_A), np.float32)
    allc = np.arange(NPAD_A)
    degc = LA["deg"]
    inva[LA["core_of"], LA["p_of"], LA["b_of"]] = np.where(
        degc > 0, 1.0 / np.maximum(degc, 1), 0.0
    ).astype(np.float32)
    # feat_clause permuted to per-core storage order (dummies -> zeros)
    fc_all = np.zeros((NCORES, P * NBLK_A, D), np.float32)
    realc = allc[:N_CLAUSE]
    rowc = LA["p_of"] * NBLK_A + LA["b_of"]
    fc_all[LA["core_of"][realc], rowc[realc]] = feat_clause

    # ---- phase B layout (literals) ----
    LB = _block_layout(lit_idx, clause_idx, N_LIT, NPAD_B, NBLK_B)
    ncols_b = LB["ncols"]
    # per-edge gather row in the assembled wh table (+1 zero row at end)
    wh_rows = NCORES * P * NBLK_A
    ZW = wh_rows
    clause_whrow = (LA["core_of"] * (P * NBLK_A) + rowc).astype(np.int32)  # [NPAD_A]
    idxb = np.full((NCORES, P, ncols_b), ZW, np.int32)
    idxb[LB["edge_core"], LB["edge_p"], LB["edge_col"]] = clause_whrow[clause_idx]
    invb = np.zeros((NCORES, P, NBLK_B), np.float32)
    degl = LB["deg"]
    invb[LB["core_of"], LB["p_of"], LB["b_of"]] = np.where(
        degl > 0, 1.0 / np.maximum(degl, 1), 0.0
    ).astype(np.float32)
    LAST_STATS["host_prep_s"] = time.time() - t_host0

    blb = np.broadcast_to(b_l2c, (P, D)).copy()
    bcb = np.broadcast_to(b_c2l, (P, D)).copy()

    # ---- launch 1 ----
    key1 = ("L1", ncols_a, tuple(LA["kb"]))
    if key1 not in _RUNNERS:
        nc1 = _build_launch1(LA["kb"], LA["col0"], ncols_a, N_LIT + 1)
        _RUNNERS[key1] = _SpmdRunner(nc1, NCORES)
    r1 = _RUNNERS[key1]
    in_maps1 = [
        {
            "lit": lit_z,
            "idxa": idxa[c],
            "inva": inva[c],
            "fc": fc_all[c],
            "Wl": W_l2c,
            "Wc": W_c2l,
            "blb": blb,
            "bcb": bcb,
        }
        for c in range(NCORES)
    ]
    r1.stage_inputs(in_maps1)
    t0 = time.time()
    res1 = r1.run_numpy()
    LAST_STATS["launch1_wall_s"] = time.time() - t0

    wh_full = np.concatenate([res1[c]["wh"] for c in range(NCORES)], axis=0)
    # deg-0 clauses: reference gives cembs=relu(0)=0 -> wh row = b_c2l
    deg0 = np.flatnonzero(degc[:N_CLAUSE] == 0)
    if len(deg0):
        wh_full[clause_whrow[deg0]] = b_c2l
    wh_z = np.vstack([wh_full, np.zeros((1, D), np.float32)])

    # ---- launch 2 ----
    key2 = ("L2", ncols_b, tuple(LB["kb"]))
    if key2 not in _RUNNERS:
        nc2 = _build_launch2(LB["kb"], LB["col0"], ncols_b, wh_rows + 1)
        _RUNNERS[key2] = _SpmdRunner(nc2, NCORES)
    r2 = _RUNNERS[key2]
    in_maps2 = [
        {"whf": wh_z, "idxb": idxb[c], "invb": invb[c]} for c in range(NCORES)
    ]
    r2.stage_inputs(in_maps2)
    t0 = time.time()
    res2 = r2.run_numpy()
    LAST_STATS["launch2_wall_s"] = time.time() - t0

    # ---- reassemble outputs ----
    h2_all = np.concatenate([res1[c]["h2"] for c in range(NCORES)], axis=0)
    h2_clause = h2_all[clause_whrow[:N_CLAUSE]]
    hl_all = np.concatenate([res2[c]["hl"] for c in range(NCORES)], axis=0)
    lit_row = (LB["core_of"] * (P * NBLK_B) + LB["p_of"] * NBLK_B + LB["b_of"]).astype(
        np.int64
    )
    h_lit = hl_all[lit_row[:N_LIT]]
    return h_lit, h2_clause


def time_launches(iters=6):
    """Re-run both staged launches to estimate device wall time (includes the
    fixed PJRT/axon dispatch overhead; subtract a null-kernel baseline for a
    cleaner kernel-only estimate)."""
    out = {}
    for key, r in _RUNNERS.items():
        ts = []
        for _ in range(iters):
            t0 = time.perf_counter()
            r.run()
            ts.append(time.perf_counter() - t0)
        out[key[0]] = min(ts)
    return out
